# revision 1
# baseline (speedup 1.0000x reference)
"""GCNII kernel v2 for 8 Trainium2 NeuronCores.

Per layer: per-edge source rows are fetched with dma_gather (fp16, 256B
descs, 4 queues) from the AllGathered node table; host-precomputed fp16
one-hot scatter tiles are streamed from HBM; the scatter matmul
(lhsT=gathered rows, rhs=one-hot) accumulates agg FEATURE-major in PSUM,
so no per-block transposes are needed. Self-loops ride as extra slots;
the initial-residual alpha*h0 term is a resident feature-major tile fused
into the PSUM->SBUF copy. Layer matrix U_l = (1-b)I + b*conv_w[l] applied
per block; outputs are produced feature-major and transposed on host.
"""
import os
import numpy as np

import concourse.bass as bass
import concourse.bacc as bacc
import concourse.mybir as mybir
import concourse.tile as tile
from concourse.bass_utils import run_bass_kernel_spmd

f32 = mybir.dt.float32
f16 = mybir.dt.float16
i16 = mybir.dt.int16
i32 = mybir.dt.int32

N = 100000
E = 1000000
IN_DIM = 256
HID = 128
LAYERS = 8
ALPHA = 0.1
THETA = 0.5
NCORES = 8
SHARD = N // NCORES          # 12500
B = 128
NBLK = -(-SHARD // B)        # 98
NQ = 4
GRP = 6
NGRP = -(-NBLK // GRP)       # 17
SP = False                   # single_packet crashes ucode at >64 descs/engine
# shard quarter-regions: table region q = concat over cores of their q-th
# quarter-shard, so the AllGather for region q can be issued as soon as the
# epilogue finishes those rows -- 4 pipelined collectives per layer.
RSZ = [3200, 3200, 3200, 2900]           # rows per core per region
RBASE = [0, 3200, 6400, 9600]
TRSZ = [r * NCORES for r in RSZ]         # table region rows
RBLK = [25, 25, 25, 23]                  # blocks per region (last block 84)
# region of local row j
def _reg_of(j):
    return np.minimum(j // 3200, 3)


def _wrap_idx(a):
    s = a.reshape(-1, 16).T.astype(np.int16)
    return np.tile(s, (8, 1))


def _build_layout(cpbq):
    """Chunk stream order: group -> quadrant -> rank -> block (banded).

    Rank-major banding within each call puts per-core all-pad chunks in a
    trailing run, which the gather ucode trims (trailing negative idxs),
    skipping their descriptors. Cells only get chunks up to their own
    cpbq[b, q] = ceil(max-over-cores count / 128).
    Returns chunk_of[b, q, r] -> stream chunk id (-1 if r >= cpbq),
    calls, total_chunks.
    """
    cr = int(cpbq.max())
    chunk_of = np.full((NBLK, NQ, cr), -1, np.int64)
    calls = []            # (g, q, chunk_base, nchunks)
    cb = 0
    for g in range(NGRP):
        blo, bhi = g * GRP, min((g + 1) * GRP, NBLK)
        for q in range(NQ):
            base = cb
            for r in range(cr):
                for b in range(blo, bhi):
                    if r < cpbq[b, q]:
                        chunk_of[b, q, r] = cb
                        cb += 1
            calls.append((g, q, base, cb - base))
    return chunk_of, calls, cb


def _build_program(cpbq, chunk_of, calls, total_chunks, maxcall, maxgrp):
    cr = int(cpbq.max())
    slots = total_chunks * 128
    nc = bacc.Bacc("TRN2", target_bir_lowering=False, debug=False,
                   num_devices=NCORES, num_swdge_queues=NQ)

    t_xT = nc.dram_tensor("xT", [IN_DIM, SHARD], f16, kind="ExternalInput")
    t_idx = nc.dram_tensor("gidx", [128, slots // 16], i16, kind="ExternalInput")
    t_ohp = nc.dram_tensor("ohp", [128, slots], f16, kind="ExternalInput")
    # weight rows: w1a w1b u0..u7 w2 wv wt b1row
    NW = 14
    t_wp = nc.dram_tensor("wpack", [NW * 128, HID], f16, kind="ExternalInput")
    t_bias = nc.dram_tensor("bias", [128, 4], f32, kind="ExternalInput")
    # per-core valid idx count per gather call (trailing -1s trimmed)
    t_nv = nc.dram_tensor("nval", [1, len(calls)], i32, kind="ExternalInput")

    t_out = nc.dram_tensor("out_s", [128, SHARD], f32, kind="ExternalOutput")
    t_xv = nc.dram_tensor("xv_s", [128, SHARD], f32, kind="ExternalOutput")
    t_xt = nc.dram_tensor("xt_s", [128, SHARD], f32, kind="ExternalOutput")

    t_tbl = [[nc.dram_tensor(f"tbl{i}_{r}", [TRSZ[r], HID], f16,
                             kind="Internal", addr_space="Shared")
              for r in range(4)] for i in range(2)]
    t_agin = [[nc.dram_tensor(f"agin{i}_{r}", [RSZ[r], HID], f16,
                              kind="Internal") for r in range(4)]
              for i in range(2)]

    RG = [list(range(NCORES))]

    with tile.TileContext(nc) as tc:
        with (
            tc.tile_pool(name="persist", bufs=1) as pp,
            tc.tile_pool(name="gp", bufs=6) as gp,
            tc.tile_pool(name="ohs", bufs=2) as ohs,
            tc.tile_pool(name="ep", bufs=6) as ep,
            tc.tile_pool(name="psa", bufs=6, space="PSUM") as psa,
            tc.tile_pool(name="pse", bufs=2, space="PSUM") as pse,
        ):
            s_idx = pp.tile([128, slots // 16], i16)
            nc.sync.dma_start(out=s_idx[:], in_=t_idx.ap())
            s_nv = pp.tile([1, len(calls)], i32, tag="nv")
            nc.sync.dma_start(out=s_nv[:], in_=t_nv.ap())
            sw = []
            for k in range(NW):
                w = pp.tile([128, HID], f16, tag=f"w{k}")
                nc.sync.dma_start(out=w[:], in_=t_wp.ap()[k * 128:(k + 1) * 128, :])
                sw.append(w)
            (w1a, w1b) = sw[0:2]
            su = sw[2:10]
            w2, wv, wt, b1row = sw[10:14]
            s_bias = pp.tile([128, 4], f32)
            nc.sync.dma_start(out=s_bias[:], in_=t_bias.ap())
            h0T = pp.tile([128, SHARD], f16, tag="h0T")
            h8T = pp.tile([128, SHARD], f16, tag="h8T")

            def nb_of(b):
                return min(B, SHARD - b * B)

            # ---------------- W1 phase (feature-major h) ----------------
            for b in range(NBLK):
                nb = nb_of(b)
                cols = slice(b * B, b * B + nb)
                xa = ep.tile([128, B], f16, tag="xa")
                xb = ep.tile([128, B], f16, tag="xb")
                nc.sync.dma_start(out=xa[:, :nb], in_=t_xT.ap()[0:128, cols])
                nc.sync.dma_start(out=xb[:, :nb], in_=t_xT.ap()[128:256, cols])
                ps = pse.tile([128, B], f32, tag="pp")
                nc.tensor.matmul(ps[:, :nb], lhsT=w1a[:], rhs=xa[:, :nb],
                                 start=True, stop=False)
                nc.tensor.matmul(ps[:, :nb], lhsT=w1b[:], rhs=xb[:, :nb],
                                 start=False, stop=True)
                hT = ep.tile([128, B], f16, tag="hT")
                nc.vector.tensor_scalar(out=hT[:, :nb], in0=ps[:, :nb],
                                        scalar1=s_bias[:, 0:1], scalar2=None,
                                        op0=mybir.AluOpType.add)
                nc.scalar.activation(out=h0T[:, cols], in_=hT[:, :nb],
                                     func=mybir.ActivationFunctionType.Relu,
                                     scale=ALPHA)
                ps_n = pse.tile([128, B], f32, tag="pp")
                nc.tensor.matmul(ps_n[:nb, :], lhsT=xa[:, :nb], rhs=w1a[:],
                                 start=True, stop=False)
                nc.tensor.matmul(ps_n[:nb, :], lhsT=xb[:, :nb], rhs=w1b[:],
                                 start=False, stop=True)
                hrow = ep.tile([128, HID], f16, tag="hrow")
                nc.vector.tensor_tensor(out=hrow[:nb, :], in0=ps_n[:nb, :],
                                        in1=b1row[:nb, :],
                                        op=mybir.AluOpType.add)
                nc.scalar.activation(out=hrow[:nb, :], in_=hrow[:nb, :],
                                     func=mybir.ActivationFunctionType.Relu)
                r = min(b // 25, 3)
                rows = slice(b * B - RBASE[r], b * B - RBASE[r] + nb)
                nc.sync.dma_start(out=t_agin[0][r].ap()[rows, :],
                                  in_=hrow[:nb, :])
            for r in range(4):
                nc.gpsimd.collective_compute(
                    "AllGather", mybir.AluOpType.bypass, replica_groups=RG,
                    ins=[t_agin[0][r].ap()], outs=[t_tbl[0][r].ap()])

            # ---------------- conv layers ----------------
            # zero the gather pool buffers once: trailing-trimmed gathers
            # leave tile tails unwritten and NaN garbage would poison psum.
            for _ in range(6):
                gz = gp.tile([128, maxcall * 128], f16, tag="gt")
                nc.gpsimd.memset(gz[:], 0.0)
            nv_reg = nc.gpsimd.alloc_register("nv_count")

            for l in range(LAYERS):
                tbl_cur = t_tbl[l % 2]
                tbl_nxt = t_tbl[(l + 1) % 2]
                agin_nxt = t_agin[(l + 1) % 2]
                for g in range(NGRP):
                    blo, bhi = g * GRP, min((g + 1) * GRP, NBLK)
                    gbase = chunk_of[blo, 0, 0]
                    gch = sum(nch for (gg, q, cb, nch) in calls if gg == g)
                    oh_g = ohs.tile([128, maxgrp * 128], f16, tag="ohg")
                    nc.sync.dma_start(
                        out=oh_g[:, :gch * 128],
                        in_=t_ohp.ap()[:, gbase * 128:(gbase + gch) * 128])
                    aggs = [psa.tile([128, B], f32, tag="agg", name="agg")
                            for _ in range(bhi - blo)]
                    gts = {}
                    for ci, (gg, q, cbase, nch) in enumerate(calls):
                        if gg != g or nch == 0:
                            continue
                        gt = gp.tile([128, maxcall * 128], f16, tag="gt")
                        nc.gpsimd.reg_load(nv_reg, s_nv[0:1, ci:ci + 1])
                        nvr = nv_reg
                        nc.gpsimd.dma_gather(
                            out_ap=gt[:, :nch * 128].rearrange(
                                "p (c f) -> p c f", f=HID),
                            in_ap=tbl_cur[q].ap(),
                            idxs_ap=s_idx[:, cbase * 8:(cbase + nch) * 8],
                            num_idxs=nch * 128,
                            num_idxs_reg=nvr,
                            elem_size=HID,
                            single_packet=SP,
                            queue_num=q,
                        )
                        gts[q] = (gt, cbase)
                    done = {b: 0 for b in range(blo, bhi)}
                    for q in range(NQ):
                        if q not in gts:
                            continue
                        gt, cbase = gts[q]
                        for r in range(cr):
                            for b in range(blo, bhi):
                                if r >= cpbq[b, q]:
                                    continue
                                ps_t = aggs[b - blo]
                                nchb = int(cpbq[b].sum())
                                ck = chunk_of[b, q, r]
                                co = ck - cbase
                                k = done[b]
                                nc.tensor.matmul(
                                    ps_t[:],
                                    lhsT=gt[:, co * 128:(co + 1) * 128],
                                    rhs=oh_g[:, (ck - gbase) * 128:
                                             (ck - gbase + 1) * 128],
                                    start=(k == 0), stop=(k == nchb - 1))
                                done[b] = k + 1
                    # epilogue (feature-major agg)
                    for b in range(blo, bhi):
                        nb = nb_of(b)
                        cols = slice(b * B, b * B + nb)
                        ps_t = aggs[b - blo]
                        aggT = ep.tile([128, B], f16, tag="aggT")
                        nc.vector.scalar_tensor_tensor(
                            out=aggT[:, :nb], in0=ps_t[:, :nb], scalar=1.0,
                            in1=h0T[:, cols],
                            op0=mybir.AluOpType.mult,
                            op1=mybir.AluOpType.add)
                        if l < LAYERS - 1:
                            ps2 = pse.tile([128, B], f32, tag="pp")
                            nc.tensor.matmul(ps2[:nb, :], lhsT=aggT[:, :nb],
                                             rhs=su[l][:], start=True, stop=True)
                            hn = ep.tile([128, HID], f16, tag="hn")
                            nc.scalar.activation(
                                out=hn[:nb, :], in_=ps2[:nb, :],
                                func=mybir.ActivationFunctionType.Relu)
                            r = min(b // 25, 3)
                            rows = slice(b * B - RBASE[r],
                                         b * B - RBASE[r] + nb)
                            nc.sync.dma_start(out=agin_nxt[r].ap()[rows, :],
                                              in_=hn[:nb, :])
                        else:
                            ps2 = pse.tile([128, B], f32, tag="pp")
                            nc.tensor.matmul(ps2[:, :nb], lhsT=su[l][:],
                                             rhs=aggT[:, :nb],
                                             start=True, stop=True)
                            nc.scalar.activation(
                                out=h8T[:, cols], in_=ps2[:, :nb],
                                func=mybir.ActivationFunctionType.Relu)
                    # pipelined region collectives: region r complete after
                    # its last block (24/49/74/97) -> groups 4/8/12/16
                    if l < LAYERS - 1:
                        for r, glast in ((0, 4), (1, 8), (2, 12), (3, 16)):
                            if g == glast:
                                nc.gpsimd.collective_compute(
                                    "AllGather", mybir.AluOpType.bypass,
                                    replica_groups=RG,
                                    ins=[agin_nxt[r].ap()],
                                    outs=[tbl_nxt[r].ap()])

            # ---------------- output heads (feature-major) ----------------
            for b in range(NBLK):
                nb = nb_of(b)
                cols = slice(b * B, b * B + nb)
                psh = pse.tile([128, B], f32, tag="pp")
                nc.tensor.matmul(psh[:, :nb], lhsT=w2[:], rhs=h8T[:, cols],
                                 start=True, stop=True)
                ob32 = ep.tile([128, B], f32, tag="ob32")
                nc.vector.tensor_scalar(out=ob32[:, :nb], in0=psh[:, :nb],
                                        scalar1=s_bias[:, 1:2], scalar2=None,
                                        op0=mybir.AluOpType.add)
                nc.sync.dma_start(out=t_out.ap()[:, cols], in_=ob32[:, :nb])
                ob16 = ep.tile([128, B], f16, tag="ob16")
                nc.scalar.activation(out=ob16[:, :nb], in_=ob32[:, :nb],
                                     func=mybir.ActivationFunctionType.Copy)
                for wmat, bcol, tdst, tg in ((wv, 2, t_xv, "xv"),
                                             (wt, 3, t_xt, "xt")):
                    ps3 = pse.tile([128, B], f32, tag="pp")
                    nc.tensor.matmul(ps3[:, :nb], lhsT=wmat[:],
                                     rhs=ob16[:, :nb], start=True, stop=True)
                    vb = ep.tile([128, B], f32, tag=tg)
                    nc.vector.tensor_scalar(out=vb[:, :nb], in0=ps3[:, :nb],
                                            scalar1=s_bias[:, bcol:bcol + 1],
                                            scalar2=None,
                                            op0=mybir.AluOpType.add)
                    nc.scalar.activation(out=vb[:, :nb], in_=vb[:, :nb],
                                         func=mybir.ActivationFunctionType.Relu)
                    nc.sync.dma_start(out=tdst.ap()[:, cols], in_=vb[:, :nb])

    nc.compile()
    return nc


def _install_profile_hook():
    """Dev-only: register the axon NTFF profiling hook (KERNEL_TRACE=1)."""
    import sys
    import types
    if "antenv.axon_hooks" in sys.modules:
        return
    try:
        mod = types.ModuleType("antenv.axon_hooks")
        state = {"hook": None}
        mod.set_axon_ntff_profile_hook = lambda h: state.__setitem__("hook", h)
        mod.get_axon_ntff_profile_hook = lambda: state["hook"]
        sys.modules["antenv.axon_hooks"] = mod
        import antenv
        antenv.axon_hooks = mod
        sys.path.insert(0, "/root/.axon_site")
        from trn_agent_boot.trn_boot import _ntff_profile_via_ctypes
        mod.set_axon_ntff_profile_hook(
            _ntff_profile_via_ctypes("/opt/axon/libaxon_pjrt.so"))
    except Exception as e:  # profiling is best-effort
        print("profile hook install failed:", e)


def kernel(**inputs):
    x = np.asarray(inputs["x"], dtype=np.float32)
    ei = np.asarray(inputs["edge_index"]).astype(np.int64)
    W1 = np.asarray(inputs["W1"], np.float32)
    b1 = np.asarray(inputs["b1"], np.float32)
    conv_w = np.asarray(inputs["conv_w"], np.float32)
    W2 = np.asarray(inputs["W2"], np.float32)
    b2 = np.asarray(inputs["b2"], np.float32)
    Wv = np.asarray(inputs["Wv"], np.float32)
    bv = np.asarray(inputs["bv"], np.float32)
    Wt = np.asarray(inputs["Wt"], np.float32)
    bt = np.asarray(inputs["bt"], np.float32)

    src = ei[0]
    dst = ei[1]
    deg = np.bincount(dst, minlength=N).astype(np.float64) + 1.0
    dinv = 1.0 / np.sqrt(deg)
    norm_e = ((1.0 - ALPHA) * dinv[src] * dinv[dst]).astype(np.float32)
    selfw_n = ((1.0 - ALPHA) * dinv * dinv).astype(np.float32)

    # slots = edges + self-loops
    a_src = np.concatenate([src, np.arange(N, dtype=np.int64)])
    a_dst = np.concatenate([dst, np.arange(N, dtype=np.int64)])
    a_w = np.concatenate([norm_e, selfw_n.astype(np.float32)])

    core = a_dst // SHARD
    blk = (a_dst % SHARD) // B
    # quadrant = region of the src node's local row; table region q holds
    # every core's q-th quarter shard, gather idx < 25600 fits int16
    s_c = a_src // SHARD
    s_j = a_src % SHARD
    quad = _reg_of(s_j)
    rsz_a = np.array(RSZ)[quad]
    rb_a = np.array(RBASE)[quad]
    g_idx = s_c * rsz_a + (s_j - rb_a)           # idx within table region

    cell = (core * NBLK + blk) * NQ + quad
    ncell = NCORES * NBLK * NQ
    cnt = np.bincount(cell, minlength=ncell).reshape(NCORES, NBLK, NQ)
    cpbq = (-(-cnt // 128)).max(axis=0)          # [NBLK, NQ]
    cr = int(cpbq.max())

    chunk_of, calls, total_chunks = _build_layout(cpbq)
    maxcall = max(nch for (_, _, _, nch) in calls)
    maxgrp = max(sum(nch for (gg, _, _, nch) in calls if gg == g)
                 for g in range(NGRP))
    slots = total_chunks * 128

    order = np.argsort(cell, kind="stable")
    cs = cell[order]
    counts = np.bincount(cell, minlength=ncell)
    pos0 = np.concatenate([[0], np.cumsum(counts)[:-1]])
    rank = np.arange(len(cs)) - pos0[cs]
    b_o = (cs // NQ) % NBLK
    q_o = cs % NQ
    c_o = cs // (NQ * NBLK)
    slot = chunk_of[b_o, q_o, rank // 128] * 128 + rank % 128

    idx_arr = np.zeros((NCORES, slots), np.int32)
    dl_arr = np.full((NCORES, slots), -1, np.int32)
    wt_arr = np.zeros((NCORES, slots), np.float32)
    idx_arr[c_o, slot] = g_idx[order]
    dl_arr[c_o, slot] = (a_dst[order] % SHARD - b_o * B)
    wt_arr[c_o, slot] = a_w[order]

    # trailing all-pad chunks per (core, call) -> idx -1 (ucode trims their
    # descriptors); a chunk (b,q,r) is all-pad for core c iff cnt <= 128*r
    TRIM = os.environ.get("KN_TRIM", "1") == "1"
    ch_cell_b = np.zeros(total_chunks, np.int64)
    ch_cell_q = np.zeros(total_chunks, np.int64)
    ch_rank = np.zeros(total_chunks, np.int64)
    for b in range(NBLK):
        for q in range(NQ):
            for r in range(cr):
                ck = chunk_of[b, q, r]
                if ck < 0:
                    continue
                ch_cell_b[ck] = b
                ch_cell_q[ck] = q
                ch_rank[ck] = r
    trimmed = 0
    nval = np.zeros((NCORES, len(calls)), np.int32)
    for c in range(NCORES):
        pad_chunk = cnt[c, ch_cell_b, ch_cell_q] <= ch_rank * 128
        for ci, (_, _, cbase, nch) in enumerate(calls):
            k = cbase + nch - 1
            while TRIM and k >= cbase and pad_chunk[k]:
                idx_arr[c, k * 128:(k + 1) * 128] = -1
                trimmed += 128
                k -= 1
            nval[c, ci] = (k - cbase + 1) * 128
    print(f"kernel_new: cr={cr} chunks={total_chunks} slots={slots} "
          f"trimmed/core={trimmed/NCORES:.0f}")

    # weights pack
    wpack = np.zeros((14 * 128, HID), np.float32)
    wpack[0:128] = W1[0:128]
    wpack[128:256] = W1[128:256]
    for l in range(LAYERS):
        beta = float(np.log(THETA / (l + 1) + 1.0))
        wpack[(2 + l) * 128:(3 + l) * 128] = \
            (1.0 - beta) * np.eye(HID, dtype=np.float32) + beta * conv_w[l]
    wpack[10 * 128:11 * 128] = W2
    wpack[11 * 128:12 * 128] = Wv
    wpack[12 * 128:13 * 128] = Wt
    wpack[13 * 128:14 * 128] = np.tile(b1, (128, 1))
    wpack16 = wpack.astype(np.float16)
    biaspk = np.stack([b1, b2, bv, bt], axis=1).astype(np.float32)

    in_maps = []
    lanes = np.arange(slots) % 128
    ck_of_slot = np.arange(slots) // 128
    for c in range(NCORES):
        # one-hot pack [128, slots]: oh[p, ck*128+j] = w if dl==j
        ohp = np.zeros((128, slots), np.float16)
        valid = dl_arr[c] >= 0
        ohp[lanes[valid], ck_of_slot[valid] * 128 + dl_arr[c][valid]] = \
            wt_arr[c][valid].astype(np.float16)
        in_maps.append({
            "xT": np.ascontiguousarray(
                x[c * SHARD:(c + 1) * SHARD].T).astype(np.float16),
            "gidx": _wrap_idx(idx_arr[c]),
            "ohp": ohp,
            "wpack": wpack16,
            "bias": biaspk,
            "nval": nval[c:c + 1],
        })

    nc = _build_program(cpbq, chunk_of, calls, total_chunks, maxcall, maxgrp)

    trace = os.environ.get("KERNEL_TRACE", "0") == "1"
    if trace:
        _install_profile_hook()
    res = run_bass_kernel_spmd(nc, in_maps, core_ids=list(range(NCORES)),
                               trace=trace)
    if trace:
        kernel.last_res = res

    out = np.concatenate([np.asarray(res.results[c]["out_s"]).T
                          for c in range(NCORES)], axis=0)
    xv = np.concatenate([np.asarray(res.results[c]["xv_s"]).T
                         for c in range(NCORES)], axis=0)
    xt = np.concatenate([np.asarray(res.results[c]["xt_s"]).T
                         for c in range(NCORES)], axis=0)
    return (out, xv, xt)

